# revision 4
# baseline (speedup 1.0000x reference)
"""BiDAF attention kernel for Trainium2, data-parallel over batch on 8 NeuronCores.

Math (per batch b, with w = [wc; wq; wm]):
    sim[i,j] = c_i@wc + q_j@wq + (c_i*wm)@q_j  =  cb_i + qb_j + s'[i,j]
    c2q      = softmax_j(sim) @ q
    q2c      = softmax_i(max_j sim) @ c            (broadcast over i)

Device strategy (softmax is shift-invariant per row, |sim| is small enough in
fp32/bf16 range that no max-subtraction is needed):
  - simT[j,i] = sum_d kT[d,j] * cT[d,i]  (bf16 matmuls, fp32 PSUM) where
    kT = (q*wm)^T and cT = c^T come pre-transposed from the host.
  - ET[j,i] = exp(simT + qb[j]) via one ACT pass per j-tile (qb is a
    per-partition bias). softmax_j(sim)[i,j] = ET[j,i] / S_i exactly.
  - c2q and row-sums S_i in one matmul group: [c2q_unnorm | S] = ET.T @ [q | 1],
    normalized on device (reciprocal + scale split across ACT/DVE).
  - row maxes: DVE max-tree over the 8 j-chunks -> mx[jj, i]; mx is DMA'd out
    and the host finishes q2c: Emax_i = max_jj mx, e2 = Emax*exp(cb),
    q2c = (e2 @ c)/sum(e2). This removes all PE transposes and q2c matmuls
    from the device hot loop.
  - software pipelining: the c2q matmuls of batch slot s-1 are interleaved
    between the sim j-tiles of slot s, so the PE never stalls waiting for the
    ACT exp of the current slot.
"""

import numpy as np

B, LC, LQ, D = 16, 1024, 1024, 256
N_CORES = 8
BPC = B // N_CORES  # batches per core

NJ = LQ // 128  # 8 j-chunks
NI = LC // 128  # 8 i-chunks
ND = D // 128   # 2 d-chunks

# of the NI c2q normalizations per slot, how many run on ACT (rest on DVE)
_NORM_ON_ACT = 0

_CACHE = {}


def build_program(repeat_inner=1, n_cores=N_CORES):
    """Build + compile the SPMD bass program (one core's view, BPC batches).

    repeat_inner > 1 repeats the whole body (for timing amplification)."""
    import concourse.bacc as bacc
    import concourse.tile as tile
    from concourse import mybir

    f32 = mybir.dt.float32
    bf16 = mybir.dt.bfloat16

    nc = bacc.Bacc(
        "TRN2",
        target_bir_lowering=False,
        debug=False,
        enable_asserts=False,
        num_devices=n_cores,
    )

    # DRAM I/O (per-core shapes)
    cT_d = nc.dram_tensor("ct", [BPC, D, LC], bf16, kind="ExternalInput").ap()
    kT_d = nc.dram_tensor("kt", [BPC, D, LQ], bf16, kind="ExternalInput").ap()
    qa_d = nc.dram_tensor("qa", [BPC, LQ, D + 2], bf16, kind="ExternalInput").ap()
    qb_d = nc.dram_tensor("qb", [BPC, 128, NJ], f32, kind="ExternalInput").ap()

    c2q_d = nc.dram_tensor("c2q", [BPC, LC, D], bf16, kind="ExternalOutput").ap()
    mx_d = nc.dram_tensor("mx", [BPC, 128, LC], bf16, kind="ExternalOutput").ap()

    # slot sequence: flat software pipeline over reps x batches
    slots = [(r, b) for r in range(repeat_inner) for b in range(BPC)]
    ns = len(slots)

    with tile.TileContext(nc) as tc:
        with (
            tc.tile_pool(name="io", bufs=2) as io_pool,
            tc.tile_pool(name="et", bufs=2) as et_pool,
            tc.tile_pool(name="tree", bufs=2) as tree_pool,
            tc.tile_pool(name="small", bufs=4) as small_pool,
            tc.tile_pool(name="outs", bufs=4) as out_pool,
            tc.tile_pool(name="psum_sim", bufs=3, space="PSUM") as sim_pool,
            tc.tile_pool(name="psum_c2q", bufs=2, space="PSUM") as c2q_pool,
        ):
            # state carried from slot s-1 to slot s for the pipelined c2q
            prev = None  # (ET tile, qa tile, b)

            def c2q_chunk(ET_p, qa_p, b_p, ic):
                pc = c2q_pool.tile([128, D + 2], f32, tag="c2q")
                for jc in range(NJ):
                    nc.tensor.matmul(
                        pc[:],
                        lhsT=ET_p[:, jc, ic * 128:(ic + 1) * 128],
                        rhs=qa_p[:, jc, :],
                        start=(jc == 0),
                        stop=(jc == NJ - 1),
                    )
                rs = small_pool.tile([128, 1], f32, tag="recip")
                nc.vector.reciprocal(rs[:], pc[:, D:D + 1])
                ot = out_pool.tile([128, D], bf16, tag="c2qo")
                if ic < _NORM_ON_ACT:
                    nc.scalar.mul(ot[:], pc[:, 0:D], rs[:])
                else:
                    nc.vector.tensor_scalar_mul(ot[:], pc[:, 0:D], rs[:])
                nc.sync.dma_start(c2q_d[b_p, ic * 128:(ic + 1) * 128, :], ot[:])

            for s in range(ns):
                rep, b = slots[s]
                # ---- loads (overlap previous slot's compute via pool bufs) ----
                kT_s = io_pool.tile([128, ND, LQ], bf16, tag="kt")
                cT_s = io_pool.tile([128, ND, LC], bf16, tag="ct")
                kT_r = kT_d[b].rearrange("(c p) n -> p c n", p=128)
                cT_r = cT_d[b].rearrange("(c p) n -> p c n", p=128)
                nc.sync.dma_start(kT_s[:, 0:1, 0:128], kT_r[:, 0:1, 0:128])
                nc.sync.dma_start(cT_s[:, 0:1, :], cT_r[:, 0:1, :])
                qb_s = io_pool.tile([128, NJ], f32, tag="qb")
                nc.sync.dma_start(qb_s[:], qb_d[b])
                nc.sync.dma_start(kT_s[:, 1:2, 0:128], kT_r[:, 1:2, 0:128])
                nc.sync.dma_start(cT_s[:, 1:2, :], cT_r[:, 1:2, :])
                nc.sync.dma_start(kT_s[:, 0:1, 128:LQ], kT_r[:, 0:1, 128:LQ])
                nc.sync.dma_start(kT_s[:, 1:2, 128:LQ], kT_r[:, 1:2, 128:LQ])
                qa_s = io_pool.tile([128, NJ, D + 2], bf16, tag="qa")
                qa_r = qa_d[b].rearrange("(c p) n -> p c n", p=128)
                nc.sync.dma_start(qa_s[:, 0:4, :], qa_r[:, 0:4, :])
                nc.sync.dma_start(qa_s[:, 4:NJ, :], qa_r[:, 4:NJ, :])

                # ---- simT + exp + running max, interleaved with prev c2q ----
                ET = et_pool.tile([128, NJ, LQ], bf16, tag="et")
                mx = tree_pool.tile([128, LC], bf16, tag="mx")
                for jt in range(NJ):
                    ps = sim_pool.tile([128, LC], f32, tag="sim")
                    for nh in range(2):
                        cols = slice(nh * 512, (nh + 1) * 512)
                        for dc in range(ND):
                            nc.tensor.matmul(
                                ps[:, cols],
                                lhsT=kT_s[:, dc, jt * 128:(jt + 1) * 128],
                                rhs=cT_s[:, dc, cols],
                                start=(dc == 0),
                                stop=(dc == ND - 1),
                            )
                    nc.scalar.activation(
                        ET[:, jt, :], ps[:],
                        mybir.ActivationFunctionType.Exp,
                        bias=qb_s[:, jt:jt + 1], scale=1.0,
                    )
                    if jt == 1:
                        nc.vector.tensor_max(mx[:], ET[:, 0, :], ET[:, 1, :])
                    elif jt > 1:
                        nc.vector.tensor_max(mx[:], mx[:], ET[:, jt, :])
                    # interleave one c2q chunk of the previous slot
                    if prev is not None:
                        c2q_chunk(prev[0], prev[1], prev[2], jt)

                nc.sync.dma_start(mx_d[b], mx[:])
                prev = (ET, qa_s, b)

            # drain: c2q of the final slot
            for ic in range(NI):
                c2q_chunk(prev[0], prev[1], prev[2], ic)

    nc.compile()
    return nc


def _host_prep(context_features, question_features, weight):
    import ml_dtypes
    bf = ml_dtypes.bfloat16

    c = np.ascontiguousarray(context_features, dtype=np.float32)
    q = np.ascontiguousarray(question_features, dtype=np.float32)
    w = np.asarray(weight, dtype=np.float32)[:, 0]
    wc, wq, wm = w[:D], w[D:2 * D], w[2 * D:]

    qb = q @ wq                       # [B, LQ]

    cT = np.ascontiguousarray(c.transpose(0, 2, 1)).astype(bf)         # [B, D, LC]
    kT = np.ascontiguousarray((q * wm).transpose(0, 2, 1)).astype(bf)  # [B, D, LQ]
    qa = np.concatenate(
        [q, np.ones((B, LQ, 1), np.float32),
         np.zeros((B, LQ, 1), np.float32)], axis=2).astype(bf)         # [B, LQ, D+2]

    qb_t = np.ascontiguousarray(
        qb.reshape(B, LQ // 128, 128).transpose(0, 2, 1))              # [B, 128, 8]

    in_maps = []
    for core in range(N_CORES):
        s = slice(core * BPC, (core + 1) * BPC)
        in_maps.append({
            "ct": cT[s], "kt": kT[s], "qa": qa[s], "qb": qb_t[s],
        })
    return in_maps


def _assemble(results, c, wc):
    c2q = np.concatenate(
        [np.asarray(r["c2q"], dtype=np.float32) for r in results], axis=0)
    mx = np.concatenate(
        [np.asarray(r["mx"], dtype=np.float32) for r in results], axis=0)

    # host q2c: Emax_i = max over the 128 jj-partitions of the chunk-maxes
    Emax = mx.max(axis=1)                                # [B, LC]
    cb = c @ wc                                          # [B, LC]
    e2 = Emax * np.exp(cb)                               # [B, LC]
    q2c_vec = np.einsum('bc,bcd->bd', e2, c) / e2.sum(axis=1)[:, None]
    q2c = np.broadcast_to(q2c_vec[:, None, :], (B, LC, D)).copy()
    return c2q, q2c


def _make_runner(nc, n_cores):
    """Compile the bass program once into a reusable sharded jax callable."""
    import jax
    import numpy as np
    from jax.sharding import Mesh, PartitionSpec
    from jax.experimental.shard_map import shard_map
    from concourse import mybir
    from concourse.bass2jax import (
        _bass_exec_p, install_neuronx_cc_hook, partition_id_tensor)

    install_neuronx_cc_hook()

    partition_name = nc.partition_id_tensor.name if nc.partition_id_tensor else None
    in_names, out_names, out_avals, zero_shapes = [], [], [], []
    for alloc in nc.m.functions[0].allocations:
        if not isinstance(alloc, mybir.MemoryLocationSet):
            continue
        name = alloc.memorylocations[0].name
        if alloc.kind == "ExternalInput":
            if name != partition_name:
                in_names.append(name)
        elif alloc.kind == "ExternalOutput":
            out_names.append(name)
            shape = tuple(alloc.tensor_shape)
            dtype = mybir.dt.np(alloc.dtype)
            out_avals.append(jax.core.ShapedArray(shape, dtype))
            zero_shapes.append((shape, dtype))
    n_params = len(in_names)
    all_names = list(in_names) + list(out_names)
    if partition_name is not None:
        all_names.append(partition_name)

    def _body(*args):
        operands = list(args)
        if partition_name is not None:
            operands.append(partition_id_tensor())
        outs = _bass_exec_p.bind(
            *operands,
            out_avals=tuple(out_avals),
            in_names=tuple(all_names),
            out_names=tuple(out_names),
            lowering_input_output_aliases=(),
            sim_require_finite=True,
            sim_require_nnan=True,
            nc=nc,
        )
        return tuple(outs)

    devices = jax.devices()[:n_cores]
    assert len(devices) == n_cores, f"need {n_cores} cores"
    mesh = Mesh(np.asarray(devices), ("core",))
    n_outs = len(out_names)
    fn = jax.jit(
        shard_map(
            _body, mesh=mesh,
            in_specs=(PartitionSpec("core"),) * (n_params + n_outs),
            out_specs=(PartitionSpec("core"),) * n_outs,
            check_rep=False),
        keep_unused=True,
    )
    sharding = jax.sharding.NamedSharding(mesh, PartitionSpec("core"))
    zeros = [
        jax.device_put(
            np.zeros((shape[0] * n_cores,) + tuple(shape[1:]), dtype), sharding)
        for shape, dtype in zero_shapes
    ]

    def run(in_maps):
        concat_in = [
            np.concatenate([np.asarray(m[name]) for m in in_maps], axis=0)
            for name in in_names
        ]
        dev_in = [jax.device_put(a, sharding) for a in concat_in]
        outs = fn(*dev_in, *zeros)
        results = []
        for cidx in range(n_cores):
            d = {}
            for name, arr in zip(out_names, outs):
                arr = np.asarray(arr)
                per = arr.shape[0] // n_cores
                d[name] = arr[cidx * per:(cidx + 1) * per]
            results.append(d)
        return results

    return run


def kernel(context_features, question_features, weight):
    if "run" not in _CACHE:
        nc = build_program()
        _CACHE["nc"] = nc
        _CACHE["run"] = _make_runner(nc, N_CORES)

    in_maps = _host_prep(context_features, question_features, weight)
    results = _CACHE["run"](in_maps)
    c = np.ascontiguousarray(context_features, dtype=np.float32)
    wc = np.asarray(weight, dtype=np.float32)[:D, 0]
    c2q, q2c = _assemble(results, c, wc)
    return c2q, q2c
